# revision 36
# baseline (speedup 1.0000x reference)
"""Transformer-XL style relative-position attention with memory + expire mask.

Full-input kernel: shards (batch 2) x (head-group 4) across 8 NeuronCores,
4 heads per core.  Each core computes its heads' attention output and a
partial out-projection; host sums the 4 head-group partials per batch.

Device dataflow (per core):
  - projections computed feature-major (qT/kT) and token-major (v)
  - rel-shift handled by writing P = q @ posT to DRAM with row pitch n+1
    and reading back at row pitch n (the pad-reshape trick)
  - attention computed transposed: D^T[jf, i] tiles; softmax denominator
    obtained by a ones-column appended to V; expire-mask folded into V rows
  - out-projection consumes y^T directly; host transposes the result back
"""

import sys

for p in ("/opt/trn_rl_repo", "/opt/pypackages"):
    if p not in sys.path:
        sys.path.insert(0, p)

from contextlib import ExitStack

import ml_dtypes
import numpy as np

import concourse.bass as bass
import concourse.tile as tile
from concourse import bacc
from concourse import mybir
from concourse.bass_utils import run_bass_kernel_spmd
from concourse.masks import make_identity
from concourse.tile import add_dep_helper

F32R = mybir.dt.float32r
F32 = mybir.dt.float32
BF16 = mybir.dt.bfloat16

DIM = 1024
N = 2048          # query length
ML = 1024         # mem length
JF = ML + N       # key/value length = 3072
DH = 64
NHC = 4           # heads per core
NP = 2            # stacked head-pairs per core
SCALE = DH ** -0.5
NTOK = JF // 128  # 24 token tiles
BLEN = N * N + 512  # shift buffer: written pitch N, read pitch N-1
IBW = 512         # i-block width in phase 3
NIB = N // IBW


def mk_ap(t, offset, pairs):
    return bass.AP(tensor=t.tensor if hasattr(t, "tensor") else t,
                   offset=offset, ap=[list(p) for p in pairs])


def build_bass():
    nc = bacc.Bacc(None, target_bir_lowering=False)

    ctxT = nc.declare_dram_parameter("ctxT", [DIM, JF], BF16, isOutput=False)
    pembT = nc.declare_dram_parameter("pembT", [DIM, N], BF16, isOutput=False)
    wq = nc.declare_dram_parameter("wq", [DIM, 256], BF16, isOutput=False)
    wk = nc.declare_dram_parameter("wk", [DIM, 256], BF16, isOutput=False)
    wv = nc.declare_dram_parameter("wv", [DIM, 256], BF16, isOutput=False)
    wp2 = nc.declare_dram_parameter("wp2", [DIM, 128], BF16, isOutput=False)
    wo = nc.declare_dram_parameter("wo", [256, DIM], BF16, isOutput=False)
    bq = nc.declare_dram_parameter("bq", [1, 256], BF16, isOutput=False)
    bk = nc.declare_dram_parameter("bk", [1, 256], BF16, isOutput=False)
    bv = nc.declare_dram_parameter("bv", [1, 256], BF16, isOutput=False)
    bp2 = nc.declare_dram_parameter("bp2", [1, 128], BF16, isOutput=False)
    em = nc.declare_dram_parameter("em", [JF, 1], F32, isOutput=False)
    outT = nc.declare_dram_parameter("outT", [DIM, N], F32, isOutput=True)

    with tile.TileContext(nc) as tc, \
            nc.allow_low_precision(reason="float32r is bitwise float32"), \
            ExitStack() as ctx:
        # ---- persistent tiles (~92K/partition) ----
        singles = ctx.enter_context(tc.tile_pool(name="singles", bufs=1))
        qT_all = singles.tile([128, NP, N], BF16, tag="qT")       # 16K/part
        kT_all = singles.tile([128, NP, JF], BF16, tag="kT")      # 24K/part
        v_all = singles.tile([128, NTOK, 4 * 65], BF16, tag="v")  # 25K/part
        pos2 = singles.tile([128, N], BF16, tag="pos2")           # 8K/part
        yT_all = singles.tile([128, NP, N], BF16, tag="yT")       # 16K/part
        ident = singles.tile([128, 128], BF16, tag="ident")
        ones_row = singles.tile([1, 512], BF16, tag="ones")
        ones64 = singles.tile([1, 64], F32, tag="ones64")
        em_sb = singles.tile([128, NTOK], F32, tag="em")
        wo_sb = singles.tile([128, 2, DIM], BF16, tag="wo")       # 8K/part

        # rel-shift scratch, one DRAM slab per head-pair ([hh, BLEN] each).
        # hh=0 half holds exp(scale*S) (multiplicative path, masked by the
        # zero wedge); hh=1 half holds raw S (additive path).
        dpool = ctx.enter_context(tc.tile_pool(name="pshift", bufs=1, space="DRAM"))
        B2 = [dpool.tile([2 * BLEN], BF16, tag=f"B{p}", name=f"B{p}")
              for p in range(NP)]
        wzero = singles.tile([128, 128], BF16, tag="wzero")

        make_identity(nc, ident)
        nc.vector.memset(ones_row, 1.0)
        nc.vector.memset(ones64, 1.0)
        nc.vector.memset(wzero, 0.0)

        # ---- phase 1: projections ----
        with tc.tile_pool(name="wpool", bufs=1) as wpool, \
                tc.tile_pool(name="proj", bufs=2) as ppool, \
                tc.tile_pool(name="projps", bufs=1, space="PSUM") as pps:
            wq_sb = wpool.tile([128, 8, 256], BF16, tag="wq")
            wk_sb = wpool.tile([128, 8, 256], BF16, tag="wk")
            wv_sb = wpool.tile([128, 8, 256], BF16, tag="wv")
            wp_sb = wpool.tile([128, 8, 128], BF16, tag="wp")
            nc.gpsimd.dma_start(out=wq_sb, in_=wq.rearrange("(kt p) f -> p kt f", p=128))
            nc.gpsimd.dma_start(out=em_sb, in_=em.rearrange("(t p) o -> p (t o)", p=128))
            nc.gpsimd.dma_start(out=wo_sb, in_=wo.rearrange("(kt p) f -> p kt f", p=128))
            nc.gpsimd.dma_start(out=wk_sb, in_=wk.rearrange("(kt p) f -> p kt f", p=128))
            nc.gpsimd.dma_start(out=wv_sb, in_=wv.rearrange("(kt p) f -> p kt f", p=128))
            nc.gpsimd.dma_start(out=wp_sb, in_=wp2.rearrange("(kt p) f -> p kt f", p=128))

            # 1a: q/k/v over the 6 token chunks of 512
            for tcn in (0, 1, 2, 3, 4, 5):
                t0 = tcn * 512
                cstr = ppool.tile([128, 8, 512], BF16, tag="ctx")
                if tcn == 0:
                    for kt in range(8):
                        nc.gpsimd.dma_start(
                            out=cstr[:, kt, :],
                            in_=ctxT[128 * kt:128 * kt + 128, t0:t0 + 512])
                else:
                    nc.gpsimd.dma_start(
                        out=cstr,
                        in_=ctxT[:, t0:t0 + 512].rearrange(
                            "(kt p) f -> p kt f", p=128))
                psk = [pps.tile([128, 512], F32, tag=f"pk{i}", name=f"psk{i}")
                       for i in range(NP)]
                in_x = t0 >= ML
                psq = [pps.tile([128, 512], F32, tag=f"pq{i}", name=f"psq{i}")
                       for i in range(NP)] if in_x else None
                psv = [pps.tile([128, 256], F32, tag=f"pv{i}", name=f"psv{i}")
                       for i in range(4)]
                for kt in range(8):
                    for pp in range(NP):
                        nc.tensor.matmul(psk[pp],
                                         (wk_sb[:, kt, 128 * pp:128 * pp + 128]),
                                         (cstr[:, kt, :]),
                                         start=(kt == 0), stop=(kt == 7))
                        if in_x:
                            nc.tensor.matmul(psq[pp],
                                             (wq_sb[:, kt, 128 * pp:128 * pp + 128]),
                                             (cstr[:, kt, :]), start=(kt == 0),
                                             stop=(kt == 7))
                    for st in range(4):
                        nc.tensor.matmul(psv[st],
                                         (cstr[:, kt, 128 * st:128 * st + 128]),
                                         (wv_sb[:, kt, :]),
                                         start=(kt == 0), stop=(kt == 7))
                # copies out of PSUM
                for pp in range(NP):
                    nc.scalar.copy(kT_all[:, pp, t0:t0 + 512], psk[pp])
                    if in_x:
                        nc.vector.tensor_copy(qT_all[:, pp, t0 - ML:t0 - ML + 512],
                                              psq[pp])
                for st in range(4):
                    tt = 4 * tcn + st
                    # em-scale + interleave into v' layout [t, 4, 65]
                    nc.vector.tensor_scalar_mul(
                        mk_ap(v_all, v_all.offset + tt * 260,
                              [v_all.ap[0], [65, 4], [1, 64]]),
                        psv[st], em_sb[:, tt:tt + 1])
                del cstr, psk, psq, psv

            # ones columns of v' (denominator lanes)
            for tt in range(NTOK):
                nc.vector.memset(
                    mk_ap(v_all, v_all.offset + tt * 260 + 64,
                          [v_all.ap[0], [65, 4]]),
                    1.0)

            # 1b: pos projection (both halves identical via duplicated Wp)
            for ncn in range(4):
                t0 = ncn * 512
                pstr = ppool.tile([128, 8, 512], BF16, tag="ctx", name="pstr")
                nc.gpsimd.dma_start(
                    out=pstr,
                    in_=pembT[:, t0:t0 + 512].rearrange("(kt p) f -> p kt f", p=128))
                psp = pps.tile([128, 512], F32, tag="pk0", name="psp")
                for kt in range(8):
                    nc.tensor.matmul(psp, (wp_sb[:, kt, :]), (pstr[:, kt, :]),
                                     start=(kt == 0), stop=(kt == 7))
                nc.scalar.copy(pos2[:, t0:t0 + 512], psp)
                del pstr, psp

        # ---- phase 2: P = q @ posT, streamed to DRAM shift buffers ----
        Bw = {}
        wedge = {}
        spool = ctx.enter_context(tc.tile_pool(name="shift", bufs=2))
        # one wedge-band write per head: rows [1, 2047], cols [0, 128) cover
        # every masked-garbage element the transposed strip reads can touch
        for h in range(NHC):
            pp, hh = divmod(h, 2)
            base = B2[pp].offset + hh * BLEN
            w1 = nc.sync.dma_start(
                out=mk_ap(B2[pp], base + N,
                          [[128 * N, 15], [N, 128], [1, 128]]),
                in_=bass.AP(tensor=wzero.tensor, offset=wzero.offset,
                            ap=[list(wzero.ap[0]), [0, 15], [1, 128]]))
            w2 = nc.sync.dma_start(
                out=mk_ap(B2[pp], base + 1921 * N,
                          [[N, 127], [1, 128]]),
                in_=wzero[:127, :])
            wedge[h] = [w1, w2]
        with tc.tile_pool(name="attn", bufs=8) as epool, \
                tc.tile_pool(name="strips", bufs=20) as stpool, \
                tc.tile_pool(name="dtps", bufs=3, space="PSUM") as dps, \
                tc.tile_pool(name="numps", bufs=1, space="PSUM") as nps, \
                tc.tile_pool(name="out", bufs=4) as opool:
            def ph2_group(pp, ti):
                """P strips for one (head-pair, i-tile): matmuls + staging
                copies + one paired B write."""
                i0 = 128 * ti
                c0 = 4 - ((ti + 1) + 3) // 4
                uneed = 128 * (15 - ti)        # exact first needed column
                wex = 2048 - uneed             # exact row width
                pbs = spool.tile([128, 2, 2048], BF16, tag="pb", name="pb")
                for c in range(c0, 4):
                    u0 = 512 * c
                    psPp = dps.tile([128, 2, IBW], F32, tag="dt",
                                    name="psPp")
                    lo = max(u0, uneed)
                    for hh in range(2):
                        sl = slice(64 * hh, 64 * hh + 64)
                        nc.tensor.matmul(psPp[:, hh, lo - u0:],
                                         (qT_all[sl, pp, i0:i0 + 128]),
                                         (pos2[sl, lo:u0 + 512]),
                                         start=True, stop=True,
                                         tile_position=(64 * hh, 0))
                    # hh=0: store exp(scale*S); hh=1: store raw S
                    nc.scalar.activation(
                        pbs[:, 0, lo - uneed:u0 - uneed + 512],
                        psPp[:, 0, lo - u0:],
                        mybir.ActivationFunctionType.Exp, scale=SCALE)
                    nc.vector.tensor_copy(
                        pbs[:, 1, lo - uneed:u0 - uneed + 512],
                        psPp[:, 1, lo - u0:])
                    del psPp
                wi = nc.sync.dma_start(
                    out=mk_ap(B2[pp], B2[pp].offset + i0 * N + uneed,
                              [[N, 128], [BLEN, 2], [1, wex]]),
                    in_=pbs[:, :, :wex])
                if ti == 15:
                    for w_ in wedge[2 * pp] + wedge[2 * pp + 1]:
                        add_dep_helper(wi.ins, w_.ins, reason="wedge WAW")
                Bw.setdefault((pp, ti), []).append(wi)

            # ---- phase 3: attention, transposed tiles ----
            NLAG = 2   # software-pipeline depth: nums(tj) emitted after
                       # matmuls of tj+NLAG so PE rides out the exp latency

            def ph3_block(pp, ib):
                i0b = IBW * ib
                tjmax = (ML + i0b + IBW - 1) // 128
                # shifted-pos tiles S^T[j, i], read transposed from B:
                # S^T[j, i] = Bflat[(i+1)(N-1) + j]
                stT = {}
                for hh in range(2):
                    for tj in range(8, tjmax + 1):
                        jt = tj - 8
                        il0 = max(0, 128 * jt - i0b)
                        st = stpool.tile([128, IBW], BF16, tag="strip",
                                         name=f"sT{hh}{tj}")
                        ri = nc.sync.dma_start(
                            out=st[:, il0:],
                            in_=mk_ap(B2[pp],
                                      B2[pp].offset + hh * BLEN
                                      + (i0b + il0 + 1) * (N - 1)
                                      + 128 * jt,
                                      [[1, 128], [N - 1, IBW - il0]]))
                        for ti in range(4 * ib, 4 * ib + 4):
                            for wi_ in Bw.get((pp, ti), []):
                                add_dep_helper(ri.ins, wi_.ins, reason="B RAW")
                        for w_ in wedge[2 * pp + hh]:
                            add_dep_helper(ri.ins, w_.ins, reason="B wedge")
                        stT[(hh, tj)] = st
                nums = [nps.tile([65, IBW], F32, tag=f"num{i}",
                                 name=f"num{i}") for i in range(2)]
                pend = []

                def emit_nums(tj_, E_, il0_):
                    for hh in range(2):
                        a = 2 * pp + hh
                        nc.tensor.matmul(nums[hh][:, il0_:],
                                         (v_all[:, tj_, 65 * a:65 * a + 65]),
                                         (E_[:, hh, il0_:]),
                                         start=(tj_ == 0),
                                         stop=(tj_ == tjmax))

                for tj in range(tjmax + 1):
                    jf0 = 128 * tj
                    jt = tj - 8
                    has_s = jt >= 0
                    il0 = max(0, jf0 - ML - i0b)
                    dtp = dps.tile([128, 2, IBW], F32, tag="dt", name="dtp")
                    for hh in range(2):
                        sl = slice(64 * hh, 64 * hh + 64)
                        nc.tensor.matmul(dtp[:, hh, il0:],
                                         (kT_all[sl, pp, jf0:jf0 + 128]),
                                         (qT_all[sl, pp,
                                                 i0b + il0:i0b + IBW]),
                                         start=True, stop=True,
                                         tile_position=(64 * hh, 0))
                    if has_s:   # additive rel-pos for hh=1, pre-exp
                        nc.vector.tensor_add(dtp[:, 1, il0:],
                                             dtp[:, 1, il0:],
                                             stT[(1, tj)][:, il0:])
                    E = epool.tile([128, 2, IBW], BF16, tag="E", name="E")
                    nc.scalar.activation(E[:, :, il0:], dtp[:, :, il0:],
                                         mybir.ActivationFunctionType.Exp,
                                         scale=SCALE)
                    del dtp
                    if has_s:   # multiplicative rel-pos for hh=0
                        nc.gpsimd.tensor_mul(E[:, 0, il0:], E[:, 0, il0:],
                                             stT[(0, tj)][:, il0:])
                    for hh in range(2):
                        # hh=1 raw-S garbage needs masking on diagonal
                        # tiles; hh=0 is masked by the zero wedge except
                        # where ti=15 P-writes overwrote it (tj == 23)
                        if (hh == 1 and jf0 + 127 > ML + i0b) or \
                                (hh == 0 and tj == 23):
                            nc.gpsimd.affine_select(
                                out=E[:, hh, il0:], in_=E[:, hh, il0:],
                                pattern=[[1, IBW - il0]],
                                base=ML + i0b + il0 - jf0,
                                channel_multiplier=-1,
                                compare_op=mybir.AluOpType.is_ge, fill=0.0)
                    pend.append((tj, E, il0))
                    if len(pend) > NLAG:
                        emit_nums(*pend.pop(0))
                    yield
                while pend:
                    emit_nums(*pend.pop(0))
                # divide by denominator row, write y^T
                for hh in range(2):
                    sl = slice(64 * hh, 64 * hh + 64)
                    rec = epool.tile([1, IBW], F32, tag="rec", name="rec")
                    nc.vector.reciprocal(rec, nums[hh][64:65, :])
                    bct = dps.tile([128, 2, IBW], F32, tag="dt", name="bct")
                    bc = bct[0:64, 0, :]
                    nc.tensor.matmul(bc, ones64, rec, start=True, stop=True)
                    ysc = epool.tile([64, IBW], F32, tag="ysc", name="ysc")
                    nc.vector.tensor_copy(ysc, nums[hh][0:64, :])
                    nc.vector.tensor_mul(yT_all[sl, pp, i0b:i0b + IBW],
                                         ysc, bc)
                    del rec, bc, bct, ysc
                del nums
                for k in list(stT):
                    del stT[k]
                if pp == 1:
                    nb = ib
                    for mo in range(8):
                        psot = dps.tile([128, 2, IBW], F32, tag="dt",
                                        name="psot")
                        pso = psot[:, 0, :]
                        for kt2 in range(2):
                            nc.tensor.matmul(
                                pso,
                                (wo_sb[:, kt2, 128 * mo:128 * mo + 128]),
                                (yT_all[:, kt2, 512 * nb:512 * nb + 512]),
                                start=(kt2 == 0), stop=(kt2 == 1))
                        ot = opool.tile([128, 512], F32, tag="ot", name="ot")
                        if mo % 2 == 0:
                            nc.vector.tensor_copy(ot, pso)
                        else:
                            nc.scalar.copy(ot, pso)
                        nc.sync.dma_start(
                            out=outT[128 * mo:128 * mo + 128,
                                     512 * nb:512 * nb + 512],
                            in_=ot)
                        del pso, psot, ot

            # schedule: ph2(pp0) fully; ph3(pp0) with ph2(pp1) groups
            # interleaved into PE dependency stalls; then ph3(pp1)
            for ti in range(16):
                ph2_group(0, ti)
            ph2q = [(1, ti) for ti in range(16)]
            for ib in range(NIB):
                for n_, _ in enumerate(ph3_block(0, ib)):
                    if n_ % 4 == 3 and ph2q:
                        ph2_group(*ph2q.pop(0))
            while ph2q:
                ph2_group(*ph2q.pop(0))
            for ib in range(NIB):
                for _ in ph3_block(1, ib):
                    pass

    nc.finalize()
    return nc


_NC_CACHE = {}


def get_nc():
    if "nc" not in _NC_CACHE:
        _NC_CACHE["nc"] = build_bass()
    return _NC_CACHE["nc"]


def make_in_maps(x, pos_emb, mem, expire_mask, Wq, bq, Wkv, bkv, Wo, Wp, bp):
    bf = ml_dtypes.bfloat16
    pembT = np.ascontiguousarray(pos_emb.T).astype(bf)
    wp2v = np.ascontiguousarray(np.concatenate([Wp, Wp], axis=1)).astype(bf)
    bp2v = np.concatenate([bp, bp]).reshape(1, 128).astype(bf)
    in_maps = []
    for c in range(8):
        bi, g = divmod(c, 4)
        cs = slice(256 * g, 256 * (g + 1))
        ctxTv = np.ascontiguousarray(
            np.concatenate([mem[bi], x[bi]], axis=0).T).astype(bf)
        em3 = np.concatenate(
            [expire_mask[bi, 0, 0], np.ones(N, np.float32)]).reshape(JF, 1)
        in_maps.append({
            "ctxT": ctxTv,
            "pembT": pembT,
            "wq": np.ascontiguousarray(Wq[:, cs]).astype(bf),
            "wk": np.ascontiguousarray(Wkv[:, :DIM][:, cs]).astype(bf),
            "wv": np.ascontiguousarray(Wkv[:, DIM:][:, cs]).astype(bf),
            "wp2": wp2v,
            "wo": np.ascontiguousarray(Wo[cs, :]).astype(bf),
            "bq": bq[cs].reshape(1, 256).astype(bf),
            "bk": bkv[:DIM][cs].reshape(1, 256).astype(bf),
            "bv": bkv[DIM:][cs].reshape(1, 256).astype(bf),
            "bp2": bp2v,
            "em": np.ascontiguousarray(em3),
        })
    return in_maps


def kernel(x, pos_emb, mem, expire_mask, Wq, bq, Wkv, bkv, Wo, bo, Wp, bp,
           _want_trace=False):
    x = np.asarray(x, np.float32)
    pos_emb = np.asarray(pos_emb, np.float32)
    mem = np.asarray(mem, np.float32)
    expire_mask = np.asarray(expire_mask, np.float32)
    Wq = np.asarray(Wq, np.float32)
    bq = np.asarray(bq, np.float32)
    Wkv = np.asarray(Wkv, np.float32)
    bkv = np.asarray(bkv, np.float32)
    Wo = np.asarray(Wo, np.float32)
    bo = np.asarray(bo, np.float32)
    Wp = np.asarray(Wp, np.float32)
    bp = np.asarray(bp, np.float32)

    for b_, nm in ((bq, "bq"), (bkv, "bkv"), (bp, "bp")):
        if np.abs(b_).max() != 0:
            raise NotImplementedError(
                f"{nm} must be zero (spec fill=zeros); the device program "
                f"folds biases out")
    in_maps = make_in_maps(x, pos_emb, mem, expire_mask, Wq, bq, Wkv, bkv,
                           Wo, Wp, bp)
    nc = get_nc()
    res = run_bass_kernel_spmd(nc, in_maps, core_ids=list(range(8)),
                               trace=_want_trace)
    out = np.zeros((x.shape[0], N, DIM), np.float32)
    for c in range(8):
        out[c // 4] += res.results[c]["outT"].T
    out += bo
    if _want_trace:
        kernel.last_exec_time_ns = res.exec_time_ns
    return out



# revision 42
# speedup vs baseline: 1.1243x; 1.1243x over previous
"""Transformer-XL style relative-position attention with memory + expire mask.

Full-input kernel: shards (batch 2) x (head-group 4) across 8 NeuronCores,
4 heads per core.  Each core computes its heads' attention output and a
partial out-projection; host sums the 4 head-group partials per batch.

Device dataflow (per core):
  - projections computed feature-major (qT/kT) and token-major (v)
  - rel-shift handled by writing P = q @ posT to DRAM with row pitch n+1
    and reading back at row pitch n (the pad-reshape trick)
  - attention computed transposed: D^T[jf, i] tiles; softmax denominator
    obtained by a ones-column appended to V; expire-mask folded into V rows
  - out-projection consumes y^T directly; host transposes the result back
"""

import sys

for p in ("/opt/trn_rl_repo", "/opt/pypackages"):
    if p not in sys.path:
        sys.path.insert(0, p)

from contextlib import ExitStack

import ml_dtypes
import numpy as np

import concourse.bass as bass
import concourse.tile as tile
from concourse import bacc
from concourse import mybir
from concourse.bass_utils import run_bass_kernel_spmd
from concourse.masks import make_identity
from concourse.tile import add_dep_helper

F32R = mybir.dt.float32r
F32 = mybir.dt.float32
BF16 = mybir.dt.bfloat16

DIM = 1024
N = 2048          # query length
ML = 1024         # mem length
JF = ML + N       # key/value length = 3072
DH = 64
NHC = 4           # heads per core
NP = 2            # stacked head-pairs per core
SCALE = DH ** -0.5
NTOK = JF // 128  # 24 token tiles
BLEN = N * N + 512  # shift buffer: written pitch N, read pitch N-1
IBW = 512         # i-block width in phase 3
NIB = N // IBW


def mk_ap(t, offset, pairs):
    return bass.AP(tensor=t.tensor if hasattr(t, "tensor") else t,
                   offset=offset, ap=[list(p) for p in pairs])


def build_bass():
    nc = bacc.Bacc(None, target_bir_lowering=False)

    ctxT = nc.declare_dram_parameter("ctxT", [DIM, JF], BF16, isOutput=False)
    pembT = nc.declare_dram_parameter("pembT", [DIM, N], BF16, isOutput=False)
    wq = nc.declare_dram_parameter("wq", [DIM, 256], BF16, isOutput=False)
    wk = nc.declare_dram_parameter("wk", [DIM, 256], BF16, isOutput=False)
    wv = nc.declare_dram_parameter("wv", [DIM, 256], BF16, isOutput=False)
    wp2 = nc.declare_dram_parameter("wp2", [DIM, 128], BF16, isOutput=False)
    wo = nc.declare_dram_parameter("wo", [256, DIM], BF16, isOutput=False)
    bq = nc.declare_dram_parameter("bq", [1, 256], BF16, isOutput=False)
    bk = nc.declare_dram_parameter("bk", [1, 256], BF16, isOutput=False)
    bv = nc.declare_dram_parameter("bv", [1, 256], BF16, isOutput=False)
    bp2 = nc.declare_dram_parameter("bp2", [1, 128], BF16, isOutput=False)
    em = nc.declare_dram_parameter("em", [JF, 1], F32, isOutput=False)
    outT = nc.declare_dram_parameter("outT", [DIM, N], F32, isOutput=True)

    with tile.TileContext(nc) as tc, \
            nc.allow_low_precision(reason="float32r is bitwise float32"), \
            ExitStack() as ctx:
        # ---- persistent tiles (~92K/partition) ----
        singles = ctx.enter_context(tc.tile_pool(name="singles", bufs=1))
        qT_all = singles.tile([128, NP, N], BF16, tag="qT")       # 16K/part
        kT_all = singles.tile([128, NP, JF], BF16, tag="kT")      # 24K/part
        v_all = singles.tile([128, NTOK, 4 * 65], BF16, tag="v")  # 25K/part
        pos2 = singles.tile([128, N], BF16, tag="pos2")           # 8K/part
        yT_all = singles.tile([128, NP, N], BF16, tag="yT")       # 16K/part
        ident = singles.tile([128, 128], BF16, tag="ident")
        ones_row = singles.tile([1, 512], BF16, tag="ones")
        ones64 = singles.tile([1, 64], F32, tag="ones64")
        em_sb = singles.tile([128, NTOK], F32, tag="em")
        wo_sb = singles.tile([128, 2, DIM], BF16, tag="wo")       # 8K/part

        # rel-shift scratch, one DRAM slab per head-pair ([hh, BLEN] each).
        # hh=0 half holds exp(scale*S) (multiplicative path, masked by the
        # zero wedge); hh=1 half holds raw S (additive path).
        dpool = ctx.enter_context(tc.tile_pool(name="pshift", bufs=1, space="DRAM"))
        B2 = [dpool.tile([2 * BLEN], BF16, tag=f"B{p}", name=f"B{p}")
              for p in range(NP)]
        wzero = singles.tile([128, 128], BF16, tag="wzero")

        make_identity(nc, ident)
        nc.vector.memset(ones_row, 1.0)
        nc.vector.memset(ones64, 1.0)
        nc.vector.memset(wzero, 0.0)

        # ---- phase 2 scaffolding (wedges, staging, PSUM pool) ----
        Bw = {}
        wedge = {}
        spool = ctx.enter_context(tc.tile_pool(name="shift", bufs=2))
        pps2 = ctx.enter_context(tc.tile_pool(name="Pps", bufs=1,
                                              space="PSUM"))
        # one wedge-band write per head: rows [1, 2047], cols [0, 128) cover
        # every masked-garbage element the transposed strip reads can touch
        for h in range(NHC):
            pp, hh = divmod(h, 2)
            base = B2[pp].offset + hh * BLEN
            w1 = nc.sync.dma_start(
                out=mk_ap(B2[pp], base + N,
                          [[128 * N, 15], [N, 128], [1, 128]]),
                in_=bass.AP(tensor=wzero.tensor, offset=wzero.offset,
                            ap=[list(wzero.ap[0]), [0, 15], [1, 128]]))
            w2 = nc.sync.dma_start(
                out=mk_ap(B2[pp], base + 1921 * N,
                          [[N, 127], [1, 128]]),
                in_=wzero[:127, :])
            wedge[h] = [w1, w2]

        def ph2_group(pp, ti):
            """P strips for one (head-pair, i-tile): matmuls + staging
            copies + one paired B write."""
            i0 = 128 * ti
            c0 = 4 - ((ti + 1) + 3) // 4
            uneed = 128 * (15 - ti)        # exact first needed column
            wex = 2048 - uneed             # exact row width
            pbs = spool.tile([128, 2, 2048], BF16, tag="pb", name="pb")
            for c in range(c0, 4):
                u0 = 512 * c
                psP = [pps2.tile([128, 512], F32, tag=f"pP{i}",
                                 name=f"psP{i}") for i in range(2)]
                lo = max(u0, uneed)
                for hh in range(2):
                    sl = slice(64 * hh, 64 * hh + 64)
                    nc.tensor.matmul(psP[hh][:, lo - u0:],
                                     (qT_all[sl, pp, i0:i0 + 128]),
                                     (pos2[sl, lo:u0 + 512]),
                                     start=True, stop=True,
                                     tile_position=(64 * hh, 0))
                # hh=0: store exp(scale*S); hh=1: store raw S
                nc.scalar.activation(
                    pbs[:, 0, lo - uneed:u0 - uneed + 512],
                    psP[0][:, lo - u0:],
                    mybir.ActivationFunctionType.Exp, scale=SCALE)
                nc.vector.tensor_copy(
                    pbs[:, 1, lo - uneed:u0 - uneed + 512],
                    psP[1][:, lo - u0:])
                del psP
            wi = nc.sync.dma_start(
                out=mk_ap(B2[pp], B2[pp].offset + i0 * N + uneed,
                          [[N, 128], [BLEN, 2], [1, wex]]),
                in_=pbs[:, :, :wex])
            if ti == 15:
                for w_ in wedge[2 * pp] + wedge[2 * pp + 1]:
                    add_dep_helper(wi.ins, w_.ins, reason="wedge WAW")
            Bw.setdefault((pp, ti), []).append(wi)

        # ---- phase 1: projections (pos-proj first so phase 2 can start
        # early; phase-2 pp0 groups interleaved into the ctx chunk loop) ----
        with tc.tile_pool(name="wpool", bufs=1) as wpool, \
                tc.tile_pool(name="proj", bufs=2) as ppool, \
                tc.tile_pool(name="projps", bufs=1, space="PSUM") as pps:
            wq_sb = wpool.tile([128, 8, 256], BF16, tag="wq")
            wk_sb = wpool.tile([128, 8, 256], BF16, tag="wk")
            wv_sb = wpool.tile([128, 8, 256], BF16, tag="wv")
            wp_sb = wpool.tile([128, 8, 128], BF16, tag="wp")
            nc.gpsimd.dma_start(out=wp_sb, in_=wp2.rearrange("(kt p) f -> p kt f", p=128))
            nc.gpsimd.dma_start(out=wq_sb, in_=wq.rearrange("(kt p) f -> p kt f", p=128))
            nc.gpsimd.dma_start(out=em_sb, in_=em.rearrange("(t p) o -> p (t o)", p=128))
            nc.gpsimd.dma_start(out=wk_sb, in_=wk.rearrange("(kt p) f -> p kt f", p=128))
            nc.gpsimd.dma_start(out=wv_sb, in_=wv.rearrange("(kt p) f -> p kt f", p=128))
            nc.gpsimd.dma_start(out=wo_sb, in_=wo.rearrange("(kt p) f -> p kt f", p=128))

            # 1b: pos projection (both halves identical via duplicated Wp)
            for ncn in range(4):
                t0 = ncn * 512
                pstr = ppool.tile([128, 8, 512], BF16, tag="ctx", name="pstr")
                nc.gpsimd.dma_start(
                    out=pstr,
                    in_=pembT[:, t0:t0 + 512].rearrange("(kt p) f -> p kt f", p=128))
                psp = pps.tile([128, 512], F32, tag="pk0", name="psp")
                for kt in range(8):
                    nc.tensor.matmul(psp, (wp_sb[:, kt, :]), (pstr[:, kt, :]),
                                     start=(kt == 0), stop=(kt == 7))
                nc.scalar.copy(pos2[:, t0:t0 + 512], psp)
                del pstr, psp

            # 1a: q/k/v over the 6 token chunks of 512
            for tcn in (0, 1, 2, 3, 4, 5):
                t0 = tcn * 512
                cstr = ppool.tile([128, 8, 512], BF16, tag="ctx")
                if tcn == 0:
                    for kt in range(8):
                        nc.gpsimd.dma_start(
                            out=cstr[:, kt, :],
                            in_=ctxT[128 * kt:128 * kt + 128, t0:t0 + 512])
                else:
                    nc.gpsimd.dma_start(
                        out=cstr,
                        in_=ctxT[:, t0:t0 + 512].rearrange(
                            "(kt p) f -> p kt f", p=128))
                psk = [pps.tile([128, 512], F32, tag=f"pk{i}", name=f"psk{i}")
                       for i in range(NP)]
                in_x = t0 >= ML
                psq = [pps.tile([128, 512], F32, tag=f"pq{i}", name=f"psq{i}")
                       for i in range(NP)] if in_x else None
                for kt in range(8):
                    for pp in range(NP):
                        nc.tensor.matmul(psk[pp],
                                         (wk_sb[:, kt, 128 * pp:128 * pp + 128]),
                                         (cstr[:, kt, :]),
                                         start=(kt == 0), stop=(kt == 7))
                        if in_x:
                            nc.tensor.matmul(psq[pp],
                                             (wq_sb[:, kt, 128 * pp:128 * pp + 128]),
                                             (cstr[:, kt, :]), start=(kt == 0),
                                             stop=(kt == 7))
                # copies out of PSUM
                for pp in range(NP):
                    nc.scalar.copy(kT_all[:, pp, t0:t0 + 512], psk[pp])
                    if in_x:
                        nc.vector.tensor_copy(qT_all[:, pp, t0 - ML:t0 - ML + 512],
                                              psq[pp])
                # v-proj: serial st-chains on 2 ping-pong tiles
                for st in range(4):
                    psv = pps.tile([128, 256], F32, tag=f"pv{st % 2}",
                                   name=f"psv{st}")
                    for kt in range(8):
                        nc.tensor.matmul(psv,
                                         (cstr[:, kt, 128 * st:128 * st + 128]),
                                         (wv_sb[:, kt, :]),
                                         start=(kt == 0), stop=(kt == 7))
                    tt = 4 * tcn + st
                    # em-scale + interleave into v' layout [t, 4, 65]
                    nc.vector.tensor_scalar_mul(
                        mk_ap(v_all, v_all.offset + tt * 260,
                              [v_all.ap[0], [65, 4], [1, 64]]),
                        psv, em_sb[:, tt:tt + 1])
                    del psv
                del cstr, psk, psq
                # phase-2 pp0 strips whose q tokens just became available
                if tcn >= 3:
                    for ti in range(4 * (tcn - 3), 4 * (tcn - 3) + 4):
                        ph2_group(0, ti)

            # ones columns of v' (denominator lanes)
            for tt in range(NTOK):
                nc.vector.memset(
                    mk_ap(v_all, v_all.offset + tt * 260 + 64,
                          [v_all.ap[0], [65, 4]]),
                    1.0)
            for ti in range(12, 16):
                ph2_group(0, ti)

        # ---- phase 3: attention, transposed tiles ----
        NLAG = 2   # software-pipeline depth: nums(tj) emitted after
                   # matmuls of tj+NLAG so PE rides out the exp latency
        with tc.tile_pool(name="attn", bufs=8) as epool, \
                tc.tile_pool(name="strips", bufs=20) as stpool, \
                tc.tile_pool(name="dtps", bufs=2, space="PSUM") as dps, \
                tc.tile_pool(name="numps", bufs=1, space="PSUM") as nps, \
                tc.tile_pool(name="out", bufs=4) as opool:
            def ph3_block(pp, ib):
                i0b = IBW * ib
                tjmax = (ML + i0b + IBW - 1) // 128
                # shifted-pos tiles S^T[j, i], read transposed from B:
                # S^T[j, i] = Bflat[(i+1)(N-1) + j]
                stT = {}
                for hh in range(2):
                    for tj in range(8, tjmax + 1):
                        jt = tj - 8
                        il0 = max(0, 128 * jt - i0b)
                        st = stpool.tile([128, IBW], BF16, tag="strip",
                                         name=f"sT{hh}{tj}")
                        ri = nc.sync.dma_start(
                            out=st[:, il0:],
                            in_=mk_ap(B2[pp],
                                      B2[pp].offset + hh * BLEN
                                      + (i0b + il0 + 1) * (N - 1)
                                      + 128 * jt,
                                      [[1, 128], [N - 1, IBW - il0]]))
                        for ti in range(4 * ib, 4 * ib + 4):
                            for wi_ in Bw.get((pp, ti), []):
                                add_dep_helper(ri.ins, wi_.ins, reason="B RAW")
                        for w_ in wedge[2 * pp + hh]:
                            add_dep_helper(ri.ins, w_.ins, reason="B wedge")
                        stT[(hh, tj)] = st
                nums = [nps.tile([65, IBW], F32, tag=f"num{i}",
                                 name=f"num{i}") for i in range(2)]
                pend = []

                def emit_nums(tj_, E_, il0_):
                    for hh in range(2):
                        a = 2 * pp + hh
                        nc.tensor.matmul(nums[hh][:, il0_:],
                                         (v_all[:, tj_, 65 * a:65 * a + 65]),
                                         (E_[:, hh, il0_:]),
                                         start=(tj_ == 0),
                                         stop=(tj_ == tjmax))

                for tj in range(tjmax + 1):
                    jf0 = 128 * tj
                    jt = tj - 8
                    has_s = jt >= 0
                    il0 = max(0, jf0 - ML - i0b)
                    dtp = dps.tile([128, 2, IBW], F32, tag="dt", name="dtp")
                    for hh in range(2):
                        sl = slice(64 * hh, 64 * hh + 64)
                        nc.tensor.matmul(dtp[:, hh, il0:],
                                         (kT_all[sl, pp, jf0:jf0 + 128]),
                                         (qT_all[sl, pp,
                                                 i0b + il0:i0b + IBW]),
                                         start=True, stop=True,
                                         tile_position=(64 * hh, 0))
                    if has_s:   # additive rel-pos for hh=1, pre-exp
                        nc.vector.tensor_add(dtp[:, 1, il0:],
                                             dtp[:, 1, il0:],
                                             stT[(1, tj)][:, il0:])
                    E = epool.tile([128, 2, IBW], BF16, tag="E", name="E")
                    nc.scalar.activation(E[:, :, il0:], dtp[:, :, il0:],
                                         mybir.ActivationFunctionType.Exp,
                                         scale=SCALE)
                    del dtp
                    if has_s:   # multiplicative rel-pos for hh=0
                        nc.gpsimd.tensor_mul(E[:, 0, il0:], E[:, 0, il0:],
                                             stT[(0, tj)][:, il0:])
                    for hh in range(2):
                        # hh=1 raw-S garbage needs masking on diagonal
                        # tiles; hh=0 is masked by the zero wedge except
                        # where ti=15 P-writes overwrote it (tj == 23)
                        if (hh == 1 and jf0 + 127 > ML + i0b) or \
                                (hh == 0 and tj == 23):
                            nc.gpsimd.affine_select(
                                out=E[:, hh, il0:], in_=E[:, hh, il0:],
                                pattern=[[1, IBW - il0]],
                                base=ML + i0b + il0 - jf0,
                                channel_multiplier=-1,
                                compare_op=mybir.AluOpType.is_ge, fill=0.0)
                    pend.append((tj, E, il0))
                    if len(pend) > NLAG:
                        emit_nums(*pend.pop(0))
                    yield
                while pend:
                    emit_nums(*pend.pop(0))
                # divide by denominator row, write y^T
                for hh in range(2):
                    sl = slice(64 * hh, 64 * hh + 64)
                    rec = epool.tile([1, IBW], F32, tag="rec", name="rec")
                    nc.vector.reciprocal(rec, nums[hh][64:65, :])
                    bc = pps2.tile([64, IBW], F32, tag="pP0", name="bc")
                    nc.tensor.matmul(bc, ones64, rec, start=True, stop=True)
                    ysc = epool.tile([64, IBW], F32, tag="ysc", name="ysc")
                    nc.vector.tensor_copy(ysc, nums[hh][0:64, :])
                    nc.vector.tensor_mul(yT_all[sl, pp, i0b:i0b + IBW],
                                         ysc, bc)
                    del rec, bc, ysc
                del nums
                for k in list(stT):
                    del stT[k]

            def outproj_emit(nb, mo):
                pso = pps2.tile([128, 512], F32, tag="pP1", name="pso")
                for kt2 in range(2):
                    nc.tensor.matmul(
                        pso,
                        (wo_sb[:, kt2, 128 * mo:128 * mo + 128]),
                        (yT_all[:, kt2, 512 * nb:512 * nb + 512]),
                        start=(kt2 == 0), stop=(kt2 == 1))
                ot = opool.tile([128, 512], F32, tag="ot", name="ot")
                if mo % 2 == 0:
                    nc.vector.tensor_copy(ot, pso)
                else:
                    nc.scalar.copy(ot, pso)
                nc.sync.dma_start(
                    out=outT[128 * mo:128 * mo + 128,
                             512 * nb:512 * nb + 512],
                    in_=ot)

            # schedule: ph3(pp0) with ph2(pp1) groups interleaved into PE
            # dependency stalls; ph3(pp1) with the previous block's
            # out-projection interleaved the same way
            ph2q = [(1, ti) for ti in range(16)]
            for ib in range(NIB):
                for n_, _ in enumerate(ph3_block(0, ib)):
                    if n_ % 4 == 3 and ph2q:
                        ph2_group(*ph2q.pop(0))
            while ph2q:
                ph2_group(*ph2q.pop(0))
            outq = []
            for ib in range(NIB):
                for n_, _ in enumerate(ph3_block(1, ib)):
                    if n_ % 3 == 2 and outq:
                        outproj_emit(*outq.pop(0))
                for mo in range(8):
                    outq.append((ib, mo))
            while outq:
                outproj_emit(*outq.pop(0))

    nc.finalize()
    return nc


_NC_CACHE = {}


def get_nc():
    if "nc" not in _NC_CACHE:
        _NC_CACHE["nc"] = build_bass()
    return _NC_CACHE["nc"]


def make_in_maps(x, pos_emb, mem, expire_mask, Wq, bq, Wkv, bkv, Wo, Wp, bp):
    bf = ml_dtypes.bfloat16
    pembT = np.ascontiguousarray(pos_emb.T).astype(bf)
    wp2v = np.ascontiguousarray(np.concatenate([Wp, Wp], axis=1)).astype(bf)
    bp2v = np.concatenate([bp, bp]).reshape(1, 128).astype(bf)
    in_maps = []
    for c in range(8):
        bi, g = divmod(c, 4)
        cs = slice(256 * g, 256 * (g + 1))
        ctxTv = np.ascontiguousarray(
            np.concatenate([mem[bi], x[bi]], axis=0).T).astype(bf)
        em3 = np.concatenate(
            [expire_mask[bi, 0, 0], np.ones(N, np.float32)]).reshape(JF, 1)
        in_maps.append({
            "ctxT": ctxTv,
            "pembT": pembT,
            "wq": np.ascontiguousarray(Wq[:, cs]).astype(bf),
            "wk": np.ascontiguousarray(Wkv[:, :DIM][:, cs]).astype(bf),
            "wv": np.ascontiguousarray(Wkv[:, DIM:][:, cs]).astype(bf),
            "wp2": wp2v,
            "wo": np.ascontiguousarray(Wo[cs, :]).astype(bf),
            "bq": bq[cs].reshape(1, 256).astype(bf),
            "bk": bkv[:DIM][cs].reshape(1, 256).astype(bf),
            "bv": bkv[DIM:][cs].reshape(1, 256).astype(bf),
            "bp2": bp2v,
            "em": np.ascontiguousarray(em3),
        })
    return in_maps


def kernel(x, pos_emb, mem, expire_mask, Wq, bq, Wkv, bkv, Wo, bo, Wp, bp,
           _want_trace=False):
    x = np.asarray(x, np.float32)
    pos_emb = np.asarray(pos_emb, np.float32)
    mem = np.asarray(mem, np.float32)
    expire_mask = np.asarray(expire_mask, np.float32)
    Wq = np.asarray(Wq, np.float32)
    bq = np.asarray(bq, np.float32)
    Wkv = np.asarray(Wkv, np.float32)
    bkv = np.asarray(bkv, np.float32)
    Wo = np.asarray(Wo, np.float32)
    bo = np.asarray(bo, np.float32)
    Wp = np.asarray(Wp, np.float32)
    bp = np.asarray(bp, np.float32)

    for b_, nm in ((bq, "bq"), (bkv, "bkv"), (bp, "bp")):
        if np.abs(b_).max() != 0:
            raise NotImplementedError(
                f"{nm} must be zero (spec fill=zeros); the device program "
                f"folds biases out")
    in_maps = make_in_maps(x, pos_emb, mem, expire_mask, Wq, bq, Wkv, bkv,
                           Wo, Wp, bp)
    nc = get_nc()
    res = run_bass_kernel_spmd(nc, in_maps, core_ids=list(range(8)),
                               trace=_want_trace)
    out = np.zeros((x.shape[0], N, DIM), np.float32)
    for c in range(8):
        out[c // 4] += res.results[c]["outT"].T
    out += bo
    if _want_trace:
        kernel.last_exec_time_ns = res.exec_time_ns
    return out

